# revision 15
# baseline (speedup 1.0000x reference)
"""Causal multi-head attention kernel for 8 Trainium2 NeuronCores.

Problem: x(4,2048,512) -> qkv proj -> 8-head causal attention -> out proj.
Sharding: core c handles batch b=c//2, heads 4*(c%2)..4*(c%2)+3.
Each core returns a partial (2048,512) output (its 4 heads' contribution
through w_out); host sums the two cores of each batch and adds b_out.

Per-core device algorithm (bf16 matmuls, fp32 psum/softmax). v2: all
non-essential work is off the tensor engine, which only runs the core
GEMMs (qkv proj, scores, PV, out proj):
  P1  QT/KT (128=2heads x 2048) per pair via w-stationary matmuls; Q/K
      biases folded into the DVE psum evacuation (per-partition add).
      V stored natural (vaug, 4 heads x 65) with a ones column for the
      softmax denominators; V bias + ones come from a host-packed bias
      tile added during the DVE evacuation (no rank-1 bias matmuls).
  P2  per head: S_T = K Q^T (k on partitions, q free) in fp32 PSUM;
      causal diag blocks masked in-place by DVE tensor_mask_reduce
      (per-partition [iota,128) keep-window, -FLT_MAX fill); exp via
      ACT (scale=1/8 folded, no max subtraction -- scores are O(9));
      out'^T/denom = [V|1]^T @ P~^T accumulated in PSUM over kk.
      Normalize: denom rows -> (16,64) tile -> one bf16 reciprocal per
      (pair,qq) -> GPSIMD partition_broadcast -> single DVE mult that
      evacuates+normalizes PSUM -> OTN.
  P3  out proj; psum evacuated via ACT copies (idle post-exp); output
      DMAd per 512-row chunk, interleaved with pair-1 attention.
"""

import os
import sys

import numpy as np

if "/opt/trn_rl_repo" not in sys.path:
    sys.path.insert(0, "/opt/trn_rl_repo")

import ml_dtypes

import concourse.bass as bass
import concourse.mybir as mybir
import concourse.tile as tile
from concourse import bacc
from concourse.bass_utils import run_bass_kernel_spmd

F32 = mybir.dt.float32
BF16 = mybir.dt.bfloat16
AF = mybir.ActivationFunctionType
ALU = mybir.AluOpType

S = 2048
D = 512
HD = 64
HPC = 4          # heads per core
NCORES = 8
SCALE = 0.125    # 1/sqrt(64)
VW = HD + 1      # 65: V plus ones column
VWS = HPC * VW   # 260

# column offsets inside the packed bf16 (128, FTOT) input
OFF_XT = 0                      # 4 tiles of (128, 2048)
OFF_WQ = OFF_XT + 4 * S         # 4 tiles of (128, 256)
OFF_WK = OFF_WQ + 4 * 256
OFF_WVA = OFF_WK + 4 * 256      # 4 tiles of (128, 260), ones cols zero
OFF_WO = OFF_WVA + 4 * VWS      # 2 tiles of (128, 512)
OFF_BQK = OFF_WO + 2 * D        # (128, 8) = bitcast fp32 (128,4):
                                #   bq p0, bq p1, bk p0, bk p1
OFF_VB = OFF_BQK + 8            # (128, 260) bias tile: bv rows + ones cols
OFF_MADD = OFF_VB + VWS         # (128, 128) additive causal mask:
                                #   0 where col >= partition, -1e30 below
OFF_ONES = OFF_MADD + 128       # row 0: ones (128) for bcast matmul lhsT
FTOT = OFF_ONES + 128


def build_nc():
    nc = bacc.Bacc("TRN2", target_bir_lowering=False, debug=False)

    wpack = nc.dram_tensor("wpack", [128, FTOT], BF16,
                           kind="ExternalInput").ap()
    out = nc.dram_tensor("out", [S, D], F32, kind="ExternalOutput").ap()

    with tile.TileContext(nc) as tc:
        _build_kernel(tc, wpack, out)
    nc.compile()
    return nc


def _build_kernel(tc, wpack, out):
    nc = tc.nc
    from contextlib import ExitStack

    ctx = ExitStack()
    with ctx:
        pers = ctx.enter_context(tc.tile_pool(name="pers", bufs=1))
        spsum = ctx.enter_context(
            tc.tile_pool(name="spsum", bufs=2, space="PSUM"))   # scores+P1/P3
        opsum = ctx.enter_context(
            tc.tile_pool(name="opsum", bufs=2, space="PSUM"))   # PV accum
        ptp = ctx.enter_context(tc.tile_pool(name="ptp", bufs=3))
        outp = ctx.enter_context(tc.tile_pool(name="outp", bufs=2))
        dnp = ctx.enter_context(tc.tile_pool(name="dnp", bufs=3))

        # ---------- P0: weights DMA first, then xT per s-chunk ----------
        wr = pers.tile([128, FTOT], BF16, tag="wr", name="wr")
        nc.sync.dma_start(wr[:, OFF_WQ:FTOT], wpack[:, OFF_WQ:FTOT])
        wp_x = wpack[:, OFF_XT:OFF_XT + 4 * S].rearrange(
            "p (d c) -> p d c", d=4)
        wr_x = wr[:, OFF_XT:OFF_XT + 4 * S].rearrange(
            "p (d c) -> p d c", d=4)
        for sc in range(4):
            nc.sync.dma_start(wr_x[:, :, 512 * sc:512 * (sc + 1)],
                              wp_x[:, :, 512 * sc:512 * (sc + 1)])

        xT_sb = [wr[:, OFF_XT + S * dc:OFF_XT + S * (dc + 1)]
                 for dc in range(4)]
        wq_sb = [wr[:, OFF_WQ + 256 * dc:OFF_WQ + 256 * (dc + 1)]
                 for dc in range(4)]
        wk_sb = [wr[:, OFF_WK + 256 * dc:OFF_WK + 256 * (dc + 1)]
                 for dc in range(4)]
        wva_sb = [wr[:, OFF_WVA + VWS * dc:OFF_WVA + VWS * (dc + 1)]
                  for dc in range(4)]
        wo_sb = [wr[:, OFF_WO + D * p:OFF_WO + D * (p + 1)]
                 for p in range(2)]
        bqk = wr[:, OFF_BQK:OFF_BQK + 8].bitcast(F32)  # [128,4] fp32
        vbias = wr[:, OFF_VB:OFF_VB + VWS]
        madd = wr[:, OFF_MADD:OFF_MADD + 128]
        ones1 = wr[0:1, OFF_ONES:OFF_ONES + 128]

        # ---------- persistent SBUF tiles ----------
        QT, KT, OTN = [], [], []
        for p in range(2):
            QT.append(pers.tile([128, S], BF16, tag=f"QT{p}", name=f"QT{p}"))
            KT.append(pers.tile([128, S], BF16, tag=f"KT{p}", name=f"KT{p}"))
            OTN.append(pers.tile([128, S], BF16, tag=f"OTN{p}",
                                 name=f"OTN{p}"))
        vaug = pers.tile([128, 16 * VWS], BF16, tag="vaug", name="vaug")

        def p1a(p):
            # Q/K projection for pair p; bias added during DVE evacuation.
            for (w_sb, bcol, dst) in ((wq_sb, bqk[:, p:p + 1], QT[p]),
                                      (wk_sb, bqk[:, 2 + p:3 + p], KT[p])):
                for sc in range(4):
                    ps = spsum.tile([128, 512], F32, tag="ps_s", name="p1ps")
                    for dc in range(4):
                        nc.tensor.matmul(
                            ps[:],
                            w_sb[dc][:, 128 * p:128 * (p + 1)],
                            xT_sb[dc][:, 512 * sc:512 * (sc + 1)],
                            start=(dc == 0), stop=(dc == 3))
                    nc.vector.tensor_scalar_add(
                        dst[:, 512 * sc:512 * (sc + 1)], ps[:], bcol)

        def p1b():
            # V projection; bias + ones column added from host tile.
            for st in range(16):
                ps = spsum.tile([128, VWS], F32, tag="ps_s", name="p1vps")
                for dc in range(4):
                    nc.tensor.matmul(
                        ps[:],
                        xT_sb[dc][:, 128 * st:128 * (st + 1)],
                        wva_sb[dc][:],
                        start=(dc == 0), stop=(dc == 3))
                nc.vector.tensor_add(
                    vaug[:, VWS * st:VWS * (st + 1)], ps[:], vbias)

        def p2(p, qq):
            ps_oo = []
            for sub in range(2):
                t = opsum.tile([VW, 512], F32, tag=f"ps_o{sub}",
                               name=f"ps_o{sub}")
                ps_oo.append(t)
            for kk in range(4 * qq + 4):
                so = max(kk * 128 - qq * 512, 0)
                diag = (kk >= 4 * qq)
                ps_s = spsum.tile([128, 1024], F32, tag="ps_s", name="ps_s")
                for sub in range(2):
                    qrows = slice(64 * sub, 64 * sub + 64)
                    nc.tensor.matmul(
                        ps_s[:, 512 * sub + so:512 * (sub + 1)],
                        KT[p][qrows, 128 * kk:128 * (kk + 1)],
                        QT[p][qrows, 512 * qq + so:512 * (qq + 1)],
                        start=True, stop=True)
                if diag:
                    # causal mask on the 128-wide diag block: add -1e30
                    # where kpos(partition) > qpos(column)
                    for sub in range(2):
                        blk = ps_s[:, 512 * sub + so:512 * sub + so + 128]
                        nc.vector.tensor_add(blk, blk, madd)
                pt = ptp.tile([128, 1024], BF16, tag="pt", name="pt")
                if so == 0:
                    nc.scalar.activation(pt[:], ps_s[:], AF.Exp, scale=SCALE)
                else:
                    # skip the fully-masked [0:so) prefix of both heads'
                    # 512-blocks with one strided-AP call
                    pss3 = ps_s[:].rearrange("p (b c) -> p b c", c=512)
                    pt3 = pt[:].rearrange("p (b c) -> p b c", c=512)
                    nc.scalar.activation(pt3[:, :, so:], pss3[:, :, so:],
                                         AF.Exp, scale=SCALE)
                for sub in range(2):
                    h = 2 * p + sub
                    nc.tensor.matmul(
                        ps_oo[sub][:, so:512],
                        vaug[:, VWS * kk + VW * h:VWS * kk + VW * h + VW],
                        pt[:, 512 * sub + so:512 * (sub + 1)],
                        start=(kk == 0), stop=(kk == 4 * qq + 3))
            # normalize this (pair, qq) chunk
            dq = dnp.tile([16, 64], F32, tag="dq", name="dq")
            for sub in range(2):
                dslot = dnp.tile([1, 512], F32, tag="ds", name="dslot")
                nc.vector.tensor_copy(dslot[:], ps_oo[sub][64:65, :])
                nc.sync.dma_start(dq[8 * sub:8 * sub + 8, :], dslot[:])
            rq = dnp.tile([16, 64], BF16, tag="rq", name="rq")
            with nc.allow_low_precision(reason="bf16 softmax recip"):
                nc.vector.reciprocal(rq[:], dq[:])
            rrow = dnp.tile([1, 1024], BF16, tag="rrow", name="rrow")
            nc.sync.dma_start(rrow[:], rq[:])
            # broadcast recip across partitions: ones column x recip row
            # (rows 0-63 read sub0's cols, rows 64-127 sub1's), evacuated
            # to SBUF by ACT so the normalize mult reads only one PSUM op
            ps_b = spsum.tile([128, 1024], F32, tag="ps_s", name="ps_b")
            for sub in range(2):
                nc.tensor.matmul(
                    ps_b[:, 512 * sub:512 * (sub + 1)], ones1,
                    rrow[0:1, 512 * sub:512 * (sub + 1)],
                    start=True, stop=True)
            rbs = dnp.tile([128, 1024], BF16, tag="rbs", name="rbs")
            nc.scalar.copy(rbs[:], ps_b[:])
            for sub in range(2):
                qrows = slice(64 * sub, 64 * sub + 64)
                nc.vector.tensor_mul(
                    OTN[p][qrows, 512 * qq:512 * (qq + 1)],
                    ps_oo[sub][0:64, :],
                    rbs[qrows, 512 * sub:512 * (sub + 1)])

        def p3(gidx):
            # out proj for seq rows [512*gidx, 512*(gidx+1))
            osb = outp.tile([128, 4 * D], F32, tag="osb", name="osb")
            for u in range(4):
                t = 4 * gidx + u
                ps_f = spsum.tile([128, 512], F32, tag="ps_s", name="p3fps")
                for p in range(2):
                    nc.tensor.matmul(
                        ps_f[:],
                        OTN[p][:, 128 * t:128 * (t + 1)],
                        wo_sb[p][:],
                        start=(p == 0), stop=(p == 1))
                nc.scalar.copy(osb[:, D * u:D * (u + 1)], ps_f[:])
            out_view = out[512 * gidx:512 * (gidx + 1), :].rearrange(
                "(u p) c -> p u c", p=128)
            nc.sync.dma_start(out_view, osb[:].rearrange(
                "p (u c) -> p u c", u=4))

        p1a(0)
        p1b()
        p1a(1)
        for qq in range(4):
            p2(0, qq)
        for qq in range(4):
            # pair-1 attention interleaved with the out-proj rows it
            # unblocks (overlaps p3/out-DMA with the attention tail)
            p2(1, qq)
            p3(qq)


def make_in_maps(x, w_qkv, b_qkv, w_out, b_out):
    x = np.asarray(x, dtype=np.float32)
    w_qkv = np.asarray(w_qkv, dtype=np.float32)
    b_qkv = np.asarray(b_qkv, dtype=np.float32)
    w_out = np.asarray(w_out, dtype=np.float32)

    wrr = w_qkv.reshape(D, 3, 8, HD)
    br = b_qkv.reshape(3, 8, HD)

    in_maps = []
    for c in range(NCORES):
        b = c // 2
        h0 = 4 * (c % 2)
        xT = np.ascontiguousarray(x[b].T)                       # (512, 2048)
        wq = wrr[:, 0, h0:h0 + 4].reshape(D, 256)
        wk = wrr[:, 1, h0:h0 + 4].reshape(D, 256)
        wv = wrr[:, 2, h0:h0 + 4].reshape(D, 256)
        bv = br[2, h0:h0 + 4].reshape(256)
        wva = np.zeros((D, VWS), dtype=np.float32)
        vbias = np.zeros((128, VWS), dtype=np.float32)
        for j in range(HPC):
            wva[:, VW * j:VW * j + HD] = wv[:, HD * j:HD * (j + 1)]
            vbias[:, VW * j:VW * j + HD] = bv[HD * j:HD * (j + 1)][None, :]
            vbias[:, VW * j + HD] = 1.0
        bq = br[0, h0:h0 + 4].reshape(256)
        bk = br[1, h0:h0 + 4].reshape(256)
        wo = w_out.reshape(8, HD, D)[h0:h0 + 4].reshape(256, D)

        wpack = np.zeros((128, FTOT), dtype=np.float32)
        for dc in range(4):
            wpack[:, OFF_XT + S * dc:OFF_XT + S * (dc + 1)] = \
                xT[128 * dc:128 * (dc + 1)]
            wpack[:, OFF_WQ + 256 * dc:OFF_WQ + 256 * (dc + 1)] = \
                wq[128 * dc:128 * (dc + 1)]
            wpack[:, OFF_WK + 256 * dc:OFF_WK + 256 * (dc + 1)] = \
                wk[128 * dc:128 * (dc + 1)]
            wpack[:, OFF_WVA + VWS * dc:OFF_WVA + VWS * (dc + 1)] = \
                wva[128 * dc:128 * (dc + 1)]
        for p in range(2):
            wpack[:, OFF_WO + D * p:OFF_WO + D * (p + 1)] = \
                wo[128 * p:128 * (p + 1)]
        wpack[:, OFF_VB:OFF_VB + VWS] = vbias

        idx = np.arange(128)
        wpack[:, OFF_MADD:OFF_MADD + 128] = np.where(
            idx[None, :] >= idx[:, None], 0.0, -1e30)
        wpack[0, OFF_ONES:OFF_ONES + 128] = 1.0

        wpack_bf = wpack.astype(ml_dtypes.bfloat16)
        # fp32 bias columns bitcast into 8 bf16 columns
        bcols = np.stack([bq[0:128], bq[128:256],
                          bk[0:128], bk[128:256]], axis=1).astype(np.float32)
        wpack_bf[:, OFF_BQK:OFF_BQK + 8] = bcols.view(np.uint16).view(
            ml_dtypes.bfloat16)

        in_maps.append({"wpack": wpack_bf})
    return in_maps


_NC_CACHE = None


def get_nc():
    global _NC_CACHE
    if _NC_CACHE is None:
        _NC_CACHE = build_nc()
    return _NC_CACHE


def run_cores(x, w_qkv, b_qkv, w_out, b_out, trace=False, trace_cores=None):
    nc = get_nc()
    in_maps = make_in_maps(x, w_qkv, b_qkv, w_out, b_out)
    br = run_bass_kernel_spmd(
        nc, in_maps, list(range(NCORES)),
        trace=trace, trace_cores=trace_cores)
    return br


def assemble(results, b_out):
    b_out = np.asarray(b_out, dtype=np.float32)
    out = np.empty((4, S, D), dtype=np.float32)
    for b in range(4):
        out[b] = results[2 * b]["out"] + results[2 * b + 1]["out"] + b_out
    return out


def kernel(x, w_qkv, b_qkv, w_out, b_out):
    br = run_cores(x, w_qkv, b_qkv, w_out, b_out, trace=False)
    return assemble(br.results, b_out)
